# revision 1
# baseline (speedup 1.0000x reference)
"""Trainium2 Bass kernel for BasicPGCBlock:
   per-pixel Gaussian smoothing (5x5, sigma = cubic(perspective)) -> dilated 3x3 conv (256->256) + bias + ReLU.

Sharding: data-parallel over batch, 1 image per NeuronCore (8 cores).

Math: the per-pixel 5x5 kernel w(u,v) = exp(-(u^2+v^2)/(2 s^2)) / Z factors through
t = exp(-1/(2 s^2)):  w(u,v) = t^(u^2+v^2) / Z, and u^2+v^2 in {0,1,2,4,5,8}.
So smoothed = sum_m c_m * S_m with c_m = t^m / Z (host-computed per-pixel planes) and
S_m = fixed 0/1 stencil sums of x, built from 11 shifted adds (separable structure).
The dilated conv is 9 taps x (2x2) 128-channel matmul tiles accumulated in PSUM.
"""

import sys

sys.path.insert(0, "/opt/trn_rl_repo")

import numpy as np
import ml_dtypes

BF16 = ml_dtypes.bfloat16

B, C, H, W = 8, 256, 96, 96
HP, WP = H + 4, W + 4          # zero-padded by 2 on each side
SLAB = 16                      # interior rows per smoothing slab
NSLAB = H // SLAB
CHUNK = 4                      # conv output rows per matmul (N = 4*96 = 384 <= 512)
NCHUNK = SLAB // CHUNK
OFFS = (-2, 0, 2)              # dilated conv offsets
MS = (0, 1, 2, 4, 5, 8)        # exponents of t present in the 5x5 kernel

_cache = {}


def _build():
    import concourse.mybir as mybir
    from concourse import bacc
    from concourse.tile import TileContext

    dt = mybir.dt
    nc = bacc.Bacc("TRN2", target_bir_lowering=False, debug=False)

    xp = nc.dram_tensor("xp", (2, 128, HP, WP), dt.bfloat16, kind="ExternalInput").ap()
    cpl = nc.dram_tensor("cpl", (6, 128, H, W), dt.bfloat16, kind="ExternalInput").ap()
    wts = nc.dram_tensor("wts", (2, 128, 9 * 2 * 128), dt.bfloat16, kind="ExternalInput").ap()
    bias = nc.dram_tensor("bias", (128, 2), dt.float32, kind="ExternalInput").ap()
    y = nc.dram_tensor("y", (2, 128, H, W), dt.float32, kind="ExternalOutput").ap()

    with TileContext(nc) as tc:
        with (
            tc.tile_pool(name="const", bufs=1) as constp,
            tc.tile_pool(name="smpool", bufs=1) as smpool,
            tc.tile_pool(name="io", bufs=3) as iop,
            tc.tile_pool(name="tmp", bufs=2) as tmp,
            tc.tile_pool(name="outp", bufs=6) as outp,
            tc.tile_pool(name="psum", bufs=8, space="PSUM") as psp,
        ):
            w_sb = constp.tile([128, 2, 9 * 2 * 128], dt.bfloat16)
            nc.sync.dma_start(out=w_sb[:, 0], in_=wts[0])
            nc.sync.dma_start(out=w_sb[:, 1], in_=wts[1])
            b_sb = constp.tile([128, 2], dt.float32)
            nc.sync.dma_start(out=b_sb, in_=bias)

            sm = [
                smpool.tile([128, HP, WP], dt.bfloat16, name=f"sm{ct}") for ct in range(2)
            ]
            nc.vector.memset(sm[0][:], 0.0)
            nc.vector.memset(sm[1][:], 0.0)

            def smooth(s):
                r0 = SLAB * s
                cp = iop.tile([128, 6, SLAB, W], dt.bfloat16, name="cp")
                for m in range(6):
                    nc.sync.dma_start(out=cp[:, m], in_=cpl[m, :, r0 : r0 + SLAB, :])
                for ct in range(2):
                    xs = iop.tile([128, SLAB + 4, WP], dt.bfloat16, name="xs")
                    nc.sync.dma_start(out=xs, in_=xp[ct, :, r0 : r0 + SLAB + 4, :])
                    P0 = xs[:, :, 2 : W + 2]
                    P1 = tmp.tile([128, SLAB + 4, W], dt.bfloat16, name="P1")
                    nc.vector.tensor_add(P1, xs[:, :, 1 : W + 1], xs[:, :, 3 : W + 3])
                    P2 = tmp.tile([128, SLAB + 4, W], dt.bfloat16, name="P2")
                    nc.vector.tensor_add(P2, xs[:, :, 0:W], xs[:, :, 4 : W + 4])

                    ctr = lambda P: P[:, 2 : SLAB + 2]
                    u1 = lambda P: P[:, 1 : SLAB + 1]
                    d1 = lambda P: P[:, 3 : SLAB + 3]
                    u2 = lambda P: P[:, 0:SLAB]
                    d2 = lambda P: P[:, 4 : SLAB + 4]

                    acc = tmp.tile([128, SLAB, W], dt.bfloat16, name="acc")
                    nc.vector.tensor_mul(acc, ctr(P0), cp[:, 0])

                    sm_out = sm[ct][:, 2 + r0 : 2 + r0 + SLAB, 2 : W + 2]

                    # m=1: S1 = (P0[h-1]+P0[h+1]) + P1[h]
                    Qa = tmp.tile([128, SLAB, W], dt.bfloat16, name="Qa")
                    nc.vector.tensor_add(Qa, u1(P0), d1(P0))
                    S = tmp.tile([128, SLAB, W], dt.bfloat16, name="S")
                    nc.vector.tensor_add(S, Qa, ctr(P1))
                    t = tmp.tile([128, SLAB, W], dt.bfloat16, name="t")
                    nc.vector.tensor_mul(t, S, cp[:, 1])
                    nc.vector.tensor_add(acc, acc, t)
                    # m=2: S2 = P1[h-1]+P1[h+1]
                    S = tmp.tile([128, SLAB, W], dt.bfloat16, name="S")
                    nc.vector.tensor_add(S, u1(P1), d1(P1))
                    t = tmp.tile([128, SLAB, W], dt.bfloat16, name="t")
                    nc.vector.tensor_mul(t, S, cp[:, 2])
                    nc.vector.tensor_add(acc, acc, t)
                    # m=4: S4 = (P0[h-2]+P0[h+2]) + P2[h]
                    Qa = tmp.tile([128, SLAB, W], dt.bfloat16, name="Qa")
                    nc.vector.tensor_add(Qa, u2(P0), d2(P0))
                    S = tmp.tile([128, SLAB, W], dt.bfloat16, name="S")
                    nc.vector.tensor_add(S, Qa, ctr(P2))
                    t = tmp.tile([128, SLAB, W], dt.bfloat16, name="t")
                    nc.vector.tensor_mul(t, S, cp[:, 3])
                    nc.vector.tensor_add(acc, acc, t)
                    # m=5: S5 = (P1[h-2]+P1[h+2]) + (P2[h-1]+P2[h+1])
                    Qa = tmp.tile([128, SLAB, W], dt.bfloat16, name="Qa")
                    nc.vector.tensor_add(Qa, u2(P1), d2(P1))
                    Qb = tmp.tile([128, SLAB, W], dt.bfloat16, name="Qb")
                    nc.vector.tensor_add(Qb, u1(P2), d1(P2))
                    S = tmp.tile([128, SLAB, W], dt.bfloat16, name="S")
                    nc.vector.tensor_add(S, Qa, Qb)
                    t = tmp.tile([128, SLAB, W], dt.bfloat16, name="t")
                    nc.vector.tensor_mul(t, S, cp[:, 4])
                    nc.vector.tensor_add(acc, acc, t)
                    # m=8: S8 = P2[h-2]+P2[h+2]
                    S = tmp.tile([128, SLAB, W], dt.bfloat16, name="S")
                    nc.vector.tensor_add(S, u2(P2), d2(P2))
                    t = tmp.tile([128, SLAB, W], dt.bfloat16, name="t")
                    nc.vector.tensor_mul(t, S, cp[:, 5])
                    nc.vector.tensor_add(sm_out, acc, t)

            def conv(s):
                r0 = SLAB * s
                for oi in range(2):
                    pcs = [
                        psp.tile([128, CHUNK, W], dt.float32, name="pc")
                        for _ in range(NCHUNK)
                    ]
                    for idx in range(18):
                        ki, q = idx // 9, idx % 9
                        dh, dw = OFFS[q // 3], OFFS[q % 3]
                        lhsT = w_sb[:, ki, (q * 2 + oi) * 128 : (q * 2 + oi + 1) * 128]
                        for k in range(NCHUNK):
                            rr = r0 + CHUNK * k
                            rhs = sm[ki][
                                :, 2 + rr + dh : 2 + rr + CHUNK + dh, 2 + dw : 2 + dw + W
                            ]
                            nc.tensor.matmul(
                                pcs[k], lhsT, rhs, start=(idx == 0), stop=(idx == 17)
                            )
                    for k in range(NCHUNK):
                        rr = r0 + CHUNK * k
                        ob = outp.tile([128, CHUNK, W], dt.float32, name="ob")
                        nc.scalar.activation(
                            ob,
                            pcs[k],
                            mybir.ActivationFunctionType.Relu,
                            bias=b_sb[:, oi : oi + 1],
                            scale=1.0,
                        )
                        nc.sync.dma_start(out=y[oi, :, rr : rr + CHUNK, :], in_=ob)

            smooth(0)
            for s in range(1, NSLAB):
                smooth(s)
                conv(s - 1)
            conv(NSLAB - 1)

    nc.compile()
    return nc


def _prep(inputs):
    x = np.asarray(inputs["x"], np.float32)
    pm = np.asarray(inputs["perspective_map"], np.float32)
    co = np.asarray(inputs["sigma_coeffs"], np.float32)
    Wc = np.asarray(inputs["conv_w"], np.float32)
    bb = np.asarray(inputs["conv_b"], np.float32)

    # per-pixel coefficient planes (host): c_m = t^m / Z, replicated over partitions
    p = pm[:, 0]  # [B,H,W]
    sigma = co[0] * p**3 + co[1] * p**2 + co[2] * p + co[3]
    sigma = np.maximum(sigma, 0.5)
    t = np.exp(-1.0 / (2.0 * sigma * sigma))
    Z = 1 + 4 * t + 4 * t**2 + 4 * t**4 + 8 * t**5 + 4 * t**8
    cpl = np.empty((B, 6, 128, H, W), BF16)
    for i, m in enumerate(MS):
        cpl[:, i] = ((t**m) / Z)[:, None, :, :].astype(BF16)

    # zero-padded bf16 input, split into two 128-channel tiles
    xpad = np.zeros((B, 2, 128, HP, WP), BF16)
    xpad[:, :, :, 2 : H + 2, 2 : W + 2] = (
        x.astype(BF16).reshape(B, 2, 128, H, W)
    )

    # conv weights: lhsT layout [ki, 128(i), q, oi, 128(o)]
    Wt = Wc.transpose(1, 0, 2, 3).astype(BF16)  # [I, O, kh, kw]
    wts = np.empty((2, 128, 9, 2, 128), BF16)
    for ki in range(2):
        for q in range(9):
            kh, kw = q // 3, q % 3
            for oi in range(2):
                wts[ki, :, q, oi, :] = Wt[
                    ki * 128 : (ki + 1) * 128, oi * 128 : (oi + 1) * 128, kh, kw
                ]
    wts = wts.reshape(2, 128, 9 * 2 * 128)
    bias_h = np.ascontiguousarray(bb.reshape(2, 128).T.astype(np.float32))  # [128, 2]

    in_maps = []
    for b in range(B):
        in_maps.append(
            {
                "xp": xpad[b],
                "cpl": cpl[b],
                "wts": wts,
                "bias": bias_h,
            }
        )
    return in_maps


def _get_nc():
    if "nc" not in _cache:
        _cache["nc"] = _build()
    return _cache["nc"]


def run(inputs, trace=False, **kw):
    from concourse.bass_utils import run_bass_kernel_spmd

    nc = _get_nc()
    in_maps = _prep(inputs)
    res = run_bass_kernel_spmd(nc, in_maps, core_ids=list(range(B)), trace=trace, **kw)
    out = np.stack([r["y"].reshape(C, H, W) for r in res.results]).astype(np.float32)
    return out, res


def kernel(**inputs):
    out, _ = run(inputs)
    return out
